# revision 8
# baseline (speedup 1.0000x reference)
"""Correlation-volume kernel for Trainium2 (8 NeuronCores, data-parallel over B).

corr[b, d, h, w] = sum_c L[b,h,w,c] * R[b,h,w-d,c], 0 <= d < 48, zero-padded w-d < 0.

Device strategy (per core = one batch):
  - Host pre-casts fp32 -> fp16 and pre-transposes rows to [H, C, W], so the
    device needs no PE transposes and reads half the bytes.
  - Per h row, banded Gram tiles G[u, w] = sum_c R^T[c,u] * L^T[c,w] in
    u-chunks of 64; two h rows packed onto the 128 PSUM partitions via
    col-tiling (tile_position=(0,64) for the odd row). Valid band window
    w in [u0, u0+110] per chunk -> 5 chunks = 508 fp32 cols, one PSUM bank.
  - One DVE copy per row-pair drains PSUM -> fp16 band block in SBUF;
    one DMA per NH rows writes the band to DRAM (1016B+ runs, full rate).
  - Host extracts the 48 diagonals (corr[d,h,w] = G[w-d, w]) while
    unsharding: host-side glue, free for the device.
"""

import os
import sys

import numpy as np

for _p in (
    "/root/.axon_site",
    "/root/.axon_site/_ro/trn_rl_repo",
    "/root/.axon_site/_ro/pypackages",
    "/opt/trn_rl_repo",
    "/opt/pypackages",
):
    if os.path.isdir(_p) and _p not in sys.path:
        sys.path.append(_p)

import concourse.bacc as bacc
import concourse.mybir as mybir
import concourse.tile as tile
from concourse.bass_utils import run_bass_kernel_spmd

B, H, W, C, D = 8, 160, 320, 128, 48
NH = 20  # h rows per DMA batch (even)
F32 = mybir.dt.float32
F16 = mybir.dt.float16

# u-chunks of 64: (u0, window width); window w in [u0, min(u0+64+47, W))
CHUNKS = [(0, 111), (64, 111), (128, 111), (192, 111), (256, 64)]
OFFS = [0, 111, 222, 333, 444]
NK = len(CHUNKS)
PSW = sum(wn for _, wn in CHUNKS)  # 508 fp32 = 2032B, fits one PSUM bank

_cache: dict = {}


def _build(h_run: int = H):
    nc = bacc.Bacc("TRN2", target_bir_lowering=False, debug=False, num_devices=B)
    L = nc.dram_tensor("L", [C, H, W], F16, kind="ExternalInput").ap()
    R = nc.dram_tensor("R", [C, H, W], F16, kind="ExternalInput").ap()
    # [(p,u), hh, j]: h = 2*hh + p; chunk k covers cols [OFFS[k], OFFS[k]+wn),
    # element [64p+i, hh, OFFS[k]+j] = G[u0+i, u0+j] = corr[j-i, 2hh+p, u0+j]
    OUT = nc.dram_tensor("OUT", [128, H // 2, PSW], F16, kind="ExternalOutput").ap()

    with tile.TileContext(nc) as tc:
        with (
            tc.tile_pool(name="loads", bufs=2) as lpool,
            tc.tile_pool(name="outbuf", bufs=2) as opool,
            tc.tile_pool(name="psg", bufs=6, space="PSUM") as psg_pool,
        ):
            for hb in range(0, h_run, NH):
                nat = {}
                for tname, src in (("L", L), ("R", R)):
                    t = lpool.tile([C, NH, W], F16, tag=f"nat{tname}")
                    nc.sync.dma_start(
                        out=t[:],
                        in_=src[:, hb : hb + NH, :],
                    )
                    nat[tname] = t

                gout = opool.tile([128, NH // 2, PSW], F16, tag="gout")

                for hp in range(NH // 2):
                    pg = psg_pool.tile([128, PSW], F32, tag="psg")
                    for p in range(2):
                        hl = 2 * hp + p
                        for (u0, wn), off in zip(CHUNKS, OFFS):
                            nc.tensor.matmul(
                                out=pg[64 * p : 64 * p + 64, off : off + wn],
                                lhsT=nat["R"][:, hl, u0 : u0 + 64],
                                rhs=nat["L"][:, hl, u0 : u0 + wn],
                                start=True,
                                stop=True,
                                tile_position=(0, 64 * p),
                            )
                    if hp % 2 == 0:
                        nc.vector.tensor_copy(out=gout[:, hp, :], in_=pg[:])
                    else:
                        nc.scalar.activation(
                            out=gout[:, hp, :],
                            in_=pg[:],
                            func=mybir.ActivationFunctionType.Copy,
                        )

                nc.scalar.dma_start(
                    out=OUT[:, hb // 2 : hb // 2 + NH // 2, :],
                    in_=gout[:],
                )

    nc.compile()
    return nc


def _get_nc(h_run: int = H):
    if h_run not in _cache:
        _cache[h_run] = _build(h_run)
    return _cache[h_run]


def _reconstruct(results) -> np.ndarray:
    """Assemble [B, D, H, W] from the per-core band blocks."""
    X = np.stack([r["OUT"] for r in results])  # [B, 128, H/2, PSW] fp16
    # partition dim 128 = (p, u) p-major -> [B, H/2, 2, u, col] -> flat last two
    Xr = X.reshape(B, 2, 64, H // 2, PSW).transpose(0, 3, 1, 2, 4)
    Xf = np.ascontiguousarray(Xr).reshape(B, H // 2, 2, 64 * PSW)
    out = np.zeros((B, D, H, W), np.float32)
    i = np.arange(64)
    for d in range(D):
        for (u0, wn), off in zip(CHUNKS, OFFS):
            nu = min(64, wn - d)
            idx = i[:nu] * (PSW + 1) + off + d
            v = Xf[:, :, :, idx]  # [B, H/2, 2, nu]
            out[:, d, :, u0 + d : u0 + d + nu] = v.reshape(B, H, nu).astype(
                np.float32
            )
    return out


def _run(L_full, R_full, h_run: int = H, trace: bool = False):
    L_full = np.asarray(L_full)
    R_full = np.asarray(R_full)
    assert L_full.shape == (B, H, W, C), L_full.shape
    nc = _get_nc(h_run)
    in_maps = [
        {
            "L": np.ascontiguousarray(
                L_full[b].astype(np.float16).transpose(2, 0, 1)
            ),
            "R": np.ascontiguousarray(
                R_full[b].astype(np.float16).transpose(2, 0, 1)
            ),
        }
        for b in range(B)
    ]
    res = run_bass_kernel_spmd(
        nc, in_maps, list(range(B)), trace=trace, trace_cores=[0] if trace else None
    )
    return _reconstruct(res.results), res


def kernel(L_corr, R_corr):
    out, _ = _run(L_corr, R_corr)
    return out


# revision 9
# speedup vs baseline: 1.1475x; 1.1475x over previous
"""Correlation-volume kernel for Trainium2 (8 NeuronCores, data-parallel over B).

corr[b, d, h, w] = sum_c L[b,h,w,c] * R[b,h,w-d,c], 0 <= d < 48, zero-padded w-d < 0.

Device strategy (per core = one batch):
  - Host pre-casts fp32 -> fp16 and pre-transposes rows to [H, C, W], so the
    device needs no PE transposes and reads half the bytes.
  - Per h row, banded Gram tiles G[u, w] = sum_c R^T[c,u] * L^T[c,w] in
    u-chunks of 64; two h rows packed onto the 128 PSUM partitions via
    col-tiling (tile_position=(0,64) for the odd row). Valid band window
    w in [u0, u0+110] per chunk -> 5 chunks = 508 fp32 cols, one PSUM bank.
  - One DVE copy per row-pair drains PSUM -> fp16 band block in SBUF;
    one DMA per NH rows writes the band to DRAM (1016B+ runs, full rate).
  - Host extracts the 48 diagonals (corr[d,h,w] = G[w-d, w]) while
    unsharding: host-side glue, free for the device.
"""

import os
import sys

import numpy as np

for _p in (
    "/root/.axon_site",
    "/root/.axon_site/_ro/trn_rl_repo",
    "/root/.axon_site/_ro/pypackages",
    "/opt/trn_rl_repo",
    "/opt/pypackages",
):
    if os.path.isdir(_p) and _p not in sys.path:
        sys.path.append(_p)

import concourse.bacc as bacc
import concourse.mybir as mybir
import concourse.tile as tile
from concourse.bass_utils import run_bass_kernel_spmd

B, H, W, C, D = 8, 160, 320, 128, 48
NH = 20  # h rows per DMA batch (even)
F32 = mybir.dt.float32
F16 = mybir.dt.float16

# u-chunks of 64: (u0, window width); window w in [u0, min(u0+64+47, W))
CHUNKS = [(0, 111), (64, 111), (128, 111), (192, 111), (256, 64)]
OFFS = [0, 111, 222, 333, 444]
NK = len(CHUNKS)
PSW = sum(wn for _, wn in CHUNKS)  # 508 fp32 = 2032B, fits one PSUM bank

_cache: dict = {}


def _build(h_run: int = H):
    nc = bacc.Bacc("TRN2", target_bir_lowering=False, debug=False, num_devices=B)
    L = nc.dram_tensor("L", [C, H, W], F16, kind="ExternalInput").ap()
    R = nc.dram_tensor("R", [C, H, W], F16, kind="ExternalInput").ap()
    # [(p,u), hh, j]: h = 2*hh + p; chunk k covers cols [OFFS[k], OFFS[k]+wn),
    # element [64p+i, hh, OFFS[k]+j] = G[u0+i, u0+j] = corr[j-i, 2hh+p, u0+j]
    OUT = nc.dram_tensor("OUT", [128, H // 2, PSW], F16, kind="ExternalOutput").ap()

    with tile.TileContext(nc) as tc:
        with (
            tc.tile_pool(name="loads", bufs=3) as lpool,
            tc.tile_pool(name="outbuf", bufs=2) as opool,
            tc.tile_pool(name="psg", bufs=6, space="PSUM") as psg_pool,
        ):
            for hb in range(0, h_run, NH):
                nat = {}
                for tname, src in (("L", L), ("R", R)):
                    t = lpool.tile([C, NH, W], F16, tag=f"nat{tname}")
                    nc.sync.dma_start(
                        out=t[:],
                        in_=src[:, hb : hb + NH, :],
                    )
                    nat[tname] = t

                gout = opool.tile([128, NH // 2, PSW], F16, tag="gout")

                for hp in range(NH // 2):
                    pg = psg_pool.tile([128, PSW], F32, tag="psg")
                    for p in range(2):
                        hl = 2 * hp + p
                        for (u0, wn), off in zip(CHUNKS, OFFS):
                            nc.tensor.matmul(
                                out=pg[64 * p : 64 * p + 64, off : off + wn],
                                lhsT=nat["R"][:, hl, u0 : u0 + 64],
                                rhs=nat["L"][:, hl, u0 : u0 + wn],
                                start=True,
                                stop=True,
                                tile_position=(0, 64 * p),
                            )
                    if hp % 2 == 0:
                        nc.vector.tensor_copy(out=gout[:, hp, :], in_=pg[:])
                    else:
                        nc.scalar.activation(
                            out=gout[:, hp, :],
                            in_=pg[:],
                            func=mybir.ActivationFunctionType.Copy,
                        )

                nc.scalar.dma_start(
                    out=OUT[:, hb // 2 : hb // 2 + NH // 2, :],
                    in_=gout[:],
                )

    nc.compile()
    return nc


def _get_nc(h_run: int = H):
    if h_run not in _cache:
        _cache[h_run] = _build(h_run)
    return _cache[h_run]


def _reconstruct(results) -> np.ndarray:
    """Assemble [B, D, H, W] from the per-core band blocks."""
    X = np.stack([r["OUT"] for r in results])  # [B, 128, H/2, PSW] fp16
    # partition dim 128 = (p, u) p-major -> [B, H/2, 2, u, col] -> flat last two
    Xr = X.reshape(B, 2, 64, H // 2, PSW).transpose(0, 3, 1, 2, 4)
    Xf = np.ascontiguousarray(Xr).reshape(B, H // 2, 2, 64 * PSW)
    out = np.zeros((B, D, H, W), np.float32)
    i = np.arange(64)
    for d in range(D):
        for (u0, wn), off in zip(CHUNKS, OFFS):
            nu = min(64, wn - d)
            idx = i[:nu] * (PSW + 1) + off + d
            v = Xf[:, :, :, idx]  # [B, H/2, 2, nu]
            out[:, d, :, u0 + d : u0 + d + nu] = v.reshape(B, H, nu).astype(
                np.float32
            )
    return out


def _run(L_full, R_full, h_run: int = H, trace: bool = False):
    L_full = np.asarray(L_full)
    R_full = np.asarray(R_full)
    assert L_full.shape == (B, H, W, C), L_full.shape
    nc = _get_nc(h_run)
    in_maps = [
        {
            "L": np.ascontiguousarray(
                L_full[b].astype(np.float16).transpose(2, 0, 1)
            ),
            "R": np.ascontiguousarray(
                R_full[b].astype(np.float16).transpose(2, 0, 1)
            ),
        }
        for b in range(B)
    ]
    res = run_bass_kernel_spmd(
        nc, in_maps, list(range(B)), trace=trace, trace_cores=[0] if trace else None
    )
    return _reconstruct(res.results), res


def kernel(L_corr, R_corr):
    out, _ = _run(L_corr, R_corr)
    return out


# revision 25
# speedup vs baseline: 1.1614x; 1.0121x over previous
"""Correlation-volume kernel for Trainium2 (8 NeuronCores, data-parallel over B).

corr[b, d, h, w] = sum_c L[b,h,w,c] * R[b,h,w-d,c], 0 <= d < 48, zero-padded w-d < 0.

Device strategy (per core = one batch):
  - Host pre-casts fp32 -> fp16 and pre-transposes rows to [H, C, W], so the
    device needs no PE transposes and reads half the bytes.
  - Per h row, banded Gram tiles G[u, w] = sum_c R^T[c,u] * L^T[c,w] in
    u-chunks of 64; two h rows packed onto the 128 PSUM partitions via
    col-tiling (tile_position=(0,64) for the odd row). Valid band window
    w in [u0, u0+110] per chunk -> 5 chunks = 508 fp32 cols, one PSUM bank.
  - One DVE copy per row-pair drains PSUM -> fp16 band block in SBUF;
    one DMA per NH rows writes the band to DRAM (1016B+ runs, full rate).
  - Host extracts the 48 diagonals (corr[d,h,w] = G[w-d, w]) while
    unsharding: host-side glue, free for the device.
"""

import os
import sys

import numpy as np

for _p in (
    "/root/.axon_site",
    "/root/.axon_site/_ro/trn_rl_repo",
    "/root/.axon_site/_ro/pypackages",
    "/opt/trn_rl_repo",
    "/opt/pypackages",
):
    if os.path.isdir(_p) and _p not in sys.path:
        sys.path.append(_p)

import concourse.bacc as bacc
import concourse.mybir as mybir
import concourse.tile as tile
from concourse.bass_utils import run_bass_kernel_spmd

B, H, W, C, D = 8, 160, 320, 128, 48
NH = 20  # max h rows per batch; tail batches shrink to cut the pipeline tail
F32 = mybir.dt.float32
F16 = mybir.dt.float16

# u-chunks of 64: (u0, window width); window w in [u0, min(u0+64+47, W))
CHUNKS = [(0, 111), (64, 111), (128, 111), (192, 111), (256, 64)]
OFFS = [0, 111, 222, 333, 444]
NK = len(CHUNKS)
PSW = sum(wn for _, wn in CHUNKS)  # 508 fp32 = 2032B, fits one PSUM bank

_cache: dict = {}


def _build(h_run: int = H):
    nc = bacc.Bacc("TRN2", target_bir_lowering=False, debug=False, num_devices=B)
    L = nc.dram_tensor("L", [C, H, W], F16, kind="ExternalInput").ap()
    R = nc.dram_tensor("R", [C, H, W], F16, kind="ExternalInput").ap()
    # [(p,u), hh, j]: h = 2*hh + p; chunk k covers cols [OFFS[k], OFFS[k]+wn),
    # element [64p+i, hh, OFFS[k]+j] = G[u0+i, u0+j] = corr[j-i, 2hh+p, u0+j]
    OUT = nc.dram_tensor("OUT", [128, H // 2, PSW], F16, kind="ExternalOutput").ap()

    with tile.TileContext(nc) as tc:
        with (
            tc.tile_pool(name="loads", bufs=4) as lpool,
            tc.tile_pool(name="tail", bufs=1) as tpool,
            tc.tile_pool(name="outbuf", bufs=3) as opool,
            tc.tile_pool(name="psg", bufs=6, space="PSUM") as psg_pool,
        ):
            if h_run == H:
                batches = [(NH, False)] * 7 + [(12, True), (8, True)]
            else:
                batches = [(min(NH, h_run), False)] * max(1, h_run // NH)
            hb = 0
            for bs, tail in batches:
                nat = {}
                for tname, src in (("L", L), ("R", R)):
                    if tail:
                        t = tpool.tile([C, bs, W], F16, tag=f"t{tname}{bs}")
                    else:
                        t = lpool.tile([C, bs, W], F16, tag=f"nat{tname}")
                    nc.sync.dma_start(
                        out=t[:],
                        in_=src[:, hb : hb + bs, :],
                    )
                    nat[tname] = t

                NP2 = bs // 4  # pairs per half-batch store
                for half in range(2):
                    gout = opool.tile([128, NH // 4, PSW], F16, tag=f"gout{half}")
                    for hq in range(NP2):
                        pg = psg_pool.tile([128, PSW], F32, tag="psg")
                        for p in range(2):
                            hl = 2 * (half * NP2 + hq) + p
                            for (u0, wn), off in zip(CHUNKS, OFFS):
                                nc.tensor.matmul(
                                    out=pg[64 * p : 64 * p + 64, off : off + wn],
                                    lhsT=nat["R"][:, hl, u0 : u0 + 64],
                                    rhs=nat["L"][:, hl, u0 : u0 + wn],
                                    start=True,
                                    stop=True,
                                    tile_position=(0, 64 * p),
                                )
                        if hq % 2 == 0:
                            nc.vector.tensor_copy(out=gout[:, hq, :], in_=pg[:])
                        else:
                            nc.scalar.activation(
                                out=gout[:, hq, :],
                                in_=pg[:],
                                func=mybir.ActivationFunctionType.Copy,
                            )

                    nc.scalar.dma_start(
                        out=OUT[
                            :,
                            hb // 2 + half * NP2 : hb // 2 + (half + 1) * NP2,
                            :,
                        ],
                        in_=gout[:, :NP2, :],
                    )
                hb += bs

    nc.compile()
    return nc


def _get_nc(h_run: int = H):
    if h_run not in _cache:
        _cache[h_run] = _build(h_run)
    return _cache[h_run]


def _reconstruct(results) -> np.ndarray:
    """Assemble [B, D, H, W] from the per-core band blocks."""
    X = np.stack([r["OUT"] for r in results])  # [B, 128, H/2, PSW] fp16
    # partition dim 128 = (p, u) p-major -> [B, H/2, 2, u, col] -> flat last two
    Xr = X.reshape(B, 2, 64, H // 2, PSW).transpose(0, 3, 1, 2, 4)
    Xf = np.ascontiguousarray(Xr).reshape(B, H // 2, 2, 64 * PSW)
    out = np.zeros((B, D, H, W), np.float32)
    i = np.arange(64)
    for d in range(D):
        for (u0, wn), off in zip(CHUNKS, OFFS):
            nu = min(64, wn - d)
            idx = i[:nu] * (PSW + 1) + off + d
            v = Xf[:, :, :, idx]  # [B, H/2, 2, nu]
            out[:, d, :, u0 + d : u0 + d + nu] = v.reshape(B, H, nu).astype(
                np.float32
            )
    return out


def _run(L_full, R_full, h_run: int = H, trace: bool = False):
    L_full = np.asarray(L_full)
    R_full = np.asarray(R_full)
    assert L_full.shape == (B, H, W, C), L_full.shape
    nc = _get_nc(h_run)
    in_maps = [
        {
            "L": np.ascontiguousarray(
                L_full[b].astype(np.float16).transpose(2, 0, 1)
            ),
            "R": np.ascontiguousarray(
                R_full[b].astype(np.float16).transpose(2, 0, 1)
            ),
        }
        for b in range(B)
    ]
    res = run_bass_kernel_spmd(
        nc, in_maps, list(range(B)), trace=trace, trace_cores=[0] if trace else None
    )
    return _reconstruct(res.results), res


def kernel(L_corr, R_corr):
    out, _ = _run(L_corr, R_corr)
    return out
